# revision 23
# baseline (speedup 1.0000x reference)
"""Multi-head attention kernel for Trainium2, sharded over 8 NeuronCores.

Problem: q,k,v [2, 4096, 256], 8 heads of d=32.  b*h = 16 head-instances
are sharded 2-per-core (core c -> batch c//4, head-pair c%4); no
cross-core communication.

Per-core algorithm (n=4096, d=32, 2 heads, fp16 operands / fp32 PSUM):

  phase 0: DMA q/k slabs [4096,64] TWICE side-by-side -> staged [128,128]
    row-chunks; PE-transpose to kT/qTt [128, n] with the head-pair
    duplicated on partitions 64-127 (d rows: h0 @0-31, h1 @32-63, dup
    @64-127).  The duplicate feeds PE-array row-tiles (64,0)/(96,0).
    PSUM->SBUF copies alternate ScalarE/DVE.  V stored as per-k-chunk
    [128, 33] tiles with a ones-column appended (softmax denominator
    rides the PV matmul for free).

  main loop (q-tiles of 512, groups of 2 k-chunks of 128):
    - S^T for 2 heads x 2 k-chunks as FOUR concurrent 32-row PE tiles
      (tile_position (0,0),(32,0),(64,0),(96,0)) -> two PSUM groups
      [128, 2x512]; 100% PE row utilization (contraction d=32).
    - exp: split across TWO engines per a per-qt schedule:
        ScalarE chunks: one ACTIVATE Exp [128,1024] PSUM->SBUF fp16.
        DVE chunks: Schraudolph pair-trick entirely on VectorE:
          i  = round(S * (scale*1024*log2e) + B)   int16  (1 elem/cyc)
          i2 = i + 512                                     (4x mode)
          P  = bitcast_fp16(i2) * 2^-0.5 + bitcast_fp16(i) (2x/4x mode)
        = exp(S*scale) * (1 +- 0.8%); the mean ratio is folded into B so
        DVE chunks match ScalarE chunks' scale.  Softmax normalization
        makes the shared bias exactly cancel.
    - O^T[33,512] += [V|1]^T P per head, 2-way col-tiled PV matmuls
      ((0,0)/(0,64)) accumulated in one PSUM bank [97, 512]; row 32/96
      collects the denominator via the ones-column.  PV skewed one group
      behind S so the PE streams S(g+1) while exp(g) runs.
    - per-q-tile epilogue (PE-transpose back, multiply by reciprocal
      denominator, DMA out) is spread across the next q-tile's groups.
"""

import numpy as np

import concourse.mybir as mybir
import concourse.tile as tile
from concourse import bacc, bass_utils
from concourse.masks import make_identity

B, N, C, H, D = 2, 4096, 256, 8, 32
NCORES = 8
HPC = 2                      # heads per core
COLS = HPC * D               # 64 per-core channel columns
P = 128                      # partitions / k-chunk
QTILE = 512                  # q columns per head per PSUM group
NKC = N // P                 # 32 k-chunks
NQT = N // QTILE             # 8 q-tiles per head
NG = NKC // 2                # 16 k-chunk groups (4-way S tiling)
SCALE = float(1.0 / np.sqrt(D))
F32 = mybir.dt.float32
BF16 = mybir.dt.float16  # fp16: same PE speed as bf16, 4x finer mantissa
I16 = mybir.dt.int16

# Schraudolph pair-trick constants (see docstring).  i = round(A*x + Bc),
# P = fp16_bits(i) + 2^-0.5 * fp16_bits(i+512) approximates
# MEANR * exp(x*SCALE); log2(MEANR) is subtracted from the bias so the
# result matches ScalarE chunks' exp() scale.
LOG2E = float(1.0 / np.log(2.0))
AEXP = float(SCALE * 1024.0 * LOG2E)
MEANR = 2.0813747       # E[pair/exp] at zero bias shift; measured, mean->1
BEXP = float(1024.0 * 15.0 - 1024.0 * np.log2(MEANR))
HALF = float(2.0 ** -0.5)

_cache = {}


def _dve_sched(dve_per_qt):
    """Bresenham spread of DVE-exp chunks over the 16 groups of a q-tile.
    Returns per-group tuple (kcA_dve, kcB_dve); kcA prefers ScalarE."""
    sched = []
    acc = 0
    for g in range(NG):
        n0 = (g * dve_per_qt) // NG
        n1 = ((g + 1) * dve_per_qt) // NG
        take = n1 - n0          # 0, 1 (or 2 when dve_per_qt > 16)
        sched.append((take >= 2, take >= 1))
    return sched


def _emit(tc, nc, q, k, v, out, heads=HPC, dve_per_qt=11, do_s=True,
          do_exp=True, do_pv=True, do_main=True, pv_skew=6):
    sched = _dve_sched(dve_per_qt)
    with tc.tile_pool(name="persist", bufs=1) as persist:
        ident = persist.tile([P, P], F32, name="ident")
        make_identity(nc, ident[:])
        identh = persist.tile([P, P], BF16, name="identh")
        nc.vector.tensor_copy(identh[:], ident[:])
        # kT/qTt duplicated layout: partitions 0-63 = heads, 64-127 = copy
        kT = persist.tile([P, N], BF16, name="kT")
        qTt = [persist.tile([P, QTILE], BF16, name=f"qTt{i}")
               for i in range(NQT)]
        # V with ones column: per head, 32 chunks of [128, 33].  vbs is
        # the same scaled by 2^-0.5 — the second Schraudolph pair piece's
        # weight, so the pair-sum rides the PV PSUM accumulation.
        vsb = persist.tile([P, HPC * NKC * (D + 1)], BF16, name="vsb")
        vbs = persist.tile([P, HPC * NKC * (D + 1)], BF16, name="vbs")

        NST = 4                      # staging quarters (whole-tile dep unit)
        CPQ = NKC // NST             # 8 row-chunks per quarter
        with tc.tile_pool(name="stage", bufs=1) as stage_pool:
            def quarter_dma(src, name):
                # each row-chunk lands contiguously via one HBM DMA, then
                # DVE converts to fp16 staged as [128, 128]-blocks: cols
                # 0-63 data, 64-127 duplicate (feeds upper PE row-tiles
                # after transpose).  fp16 makes the PE transposes 1 cyc/row
                # instead of 2.
                tiles = []
                for g in range(NST):
                    stf = stage_pool.tile([P, CPQ * COLS], F32,
                                          name=f"{name}f{g}")
                    st = stage_pool.tile([P, CPQ * 2 * COLS], BF16,
                                         name=f"{name}{g}")
                    sv = st[:].rearrange("p (i two d) -> p i two d", two=2,
                                         d=COLS)
                    svf = stf[:].rearrange("p (i d) -> p i d", d=COLS)
                    nc.sync.dma_start(
                        svf,
                        src.rearrange("(i p) d -> i p d", p=P)[
                            g * CPQ:(g + 1) * CPQ].rearrange("i p d -> p i d"),
                    )
                    # fp32->fp16 cast on the otherwise-idle GpSimd; the
                    # duplicate halves via a SBUF->SBUF DMA (off-engine)
                    nc.gpsimd.tensor_copy(sv[:, :, 0, :], svf)
                    nc.sync.dma_start(sv[:, :, 1, :], sv[:, :, 0, :])
                    tiles.append(st)
                return tiles

            # DMA queue order: k first (needed in full before any S),
            # then q quarter by quarter, V last (only PV needs it).
            kst = quarter_dma(k, "kst")
            qst = quarter_dma(q, "qst")
            vstage = stage_pool.tile([P, NKC * COLS], F32, name="vstage")
            nc.sync.dma_start(
                vstage[:].rearrange("p (i d) -> p i d", d=COLS),
                v.rearrange("(i p) d -> p i d", p=P),
            )

            with tc.tile_pool(name="tp", bufs=4, space="PSUM") as tp:
                # batches of 4 row-chunk transposes -> one [128,512] copy;
                # copies alternate DVE / ScalarE (both idle in phase 0).
                def transpose_in(tiles, dst_of_batch, name):
                    for j in range(NKC // 4):
                        pt = tp.tile([P, 4 * P], BF16, tag="pt")
                        for jj in range(4):
                            i = 4 * j + jj
                            g, ii = divmod(i, CPQ)
                            nc.tensor.transpose(
                                pt[:, jj * P:(jj + 1) * P],
                                tiles[g][:, ii * 2 * COLS:(ii + 1) * 2 * COLS],
                                identh[:],
                            )
                        dst = dst_of_batch(j)
                        if j % 2 == 0:
                            nc.vector.tensor_copy(dst, pt[:])
                        else:
                            nc.scalar.activation(
                                dst, pt[:],
                                mybir.ActivationFunctionType.Copy,
                            )

                transpose_in(kst, lambda j: kT[:, j * 512:(j + 1) * 512],
                             "kT")
                transpose_in(qst, lambda j: qTt[j][:], "qT")

                vv = vsb[:].rearrange("p (hh i e) -> p hh i e",
                                      hh=HPC, e=D + 1)
                vst = vstage[:].rearrange("p (i d) -> p i d", d=COLS)
                for hh in range(HPC):
                    nc.vector.tensor_copy(
                        vv[:, hh, :, 0:D], vst[:, :, hh * D:(hh + 1) * D]
                    )
                onescol = persist.tile([P, HPC * NKC], F32, name="onescol")
                nc.vector.memset(onescol[:], 1.0)
                nc.vector.tensor_copy(
                    vv[:, :, :, D],
                    onescol[:].rearrange("p (hh i) -> p hh i", hh=HPC),
                )
                nc.vector.tensor_scalar_mul(vbs[:], vsb[:], HALF)

        if not do_main:
            return
        with (
            tc.tile_pool(name="ps", bufs=3, space="PSUM") as ps_pool,
            tc.tile_pool(name="po", bufs=2, space="PSUM") as po_pool,
            tc.tile_pool(name="pexp", bufs=8) as pexp_pool,
            tc.tile_pool(name="pint", bufs=4) as pint_pool,
            tc.tile_pool(name="pint2", bufs=4) as pint2_pool,
            tc.tile_pool(name="osb", bufs=2) as osb_pool,
            tc.tile_pool(name="rec", bufs=3) as rec_pool,
            tc.tile_pool(name="outsb", bufs=3) as outsb_pool,
        ):
            if not do_s:
                ps_fix = ps_pool.tile([P, HPC * QTILE], F32, tag="ps")
                nc.vector.memset(ps_fix[:], 0.25)
            if not do_exp:
                pexp_fix = pexp_pool.tile([P, HPC * QTILE], BF16, tag="pexp")
                nc.vector.memset(pexp_fix[:], 0.5)

            def emit_s_group(qt, g):
                """4-way row-tiled S^T for chunks (2g, 2g+1), both heads."""
                if not do_s:
                    return (ps_fix, ps_fix)
                psA = ps_pool.tile([P, HPC * QTILE], F32, tag="ps")
                psB = ps_pool.tile([P, HPC * QTILE], F32, tag="ps")
                kcA, kcB = 2 * g, 2 * g + 1
                for hh in range(heads):
                    rp = slice(D * hh, D * (hh + 1))
                    nc.tensor.matmul(
                        psA[:, hh * QTILE:(hh + 1) * QTILE],
                        lhsT=kT[rp, kcA * P:(kcA + 1) * P],
                        rhs=qTt[qt][rp, :],
                        start=True, stop=True,
                        tile_position=(D * hh, 0),
                    )
                for hh in range(heads):
                    rp = slice(64 + D * hh, 64 + D * (hh + 1))
                    nc.tensor.matmul(
                        psB[:, hh * QTILE:(hh + 1) * QTILE],
                        lhsT=kT[rp, kcB * P:(kcB + 1) * P],
                        rhs=qTt[qt][rp, :],
                        start=True, stop=True,
                        tile_position=(64 + D * hh, 0),
                    )
                return (psA, psB)

            def emit_exp(ps, use_dve):
                """exp of one [128, 2*512] S chunk.  ScalarE: one fp16 tile.
                DVE: two int16 Schraudolph pieces (summed later on the PE)."""
                if not do_exp:
                    return ((pexp_fix,), False)
                if not use_dve:
                    pexp = pexp_pool.tile([P, HPC * QTILE], BF16, tag="pexp")
                    nc.scalar.activation(
                        pexp[:], ps[:], mybir.ActivationFunctionType.Exp,
                        scale=SCALE,
                    )
                    return ((pexp,), False)
                pint = pint_pool.tile([P, HPC * QTILE], I16, tag="pint")
                nc.vector.tensor_scalar(
                    pint[:], ps[:], AEXP, BEXP,
                    mybir.AluOpType.mult, mybir.AluOpType.add,
                )
                pint2 = pint2_pool.tile([P, HPC * QTILE], I16, tag="pint2")
                nc.vector.tensor_scalar_add(pint2[:], pint[:], 512)
                return ((pint, pint2), True)

            def emit_pv(poc, pieces, kc, first, last):
                if not do_pv:
                    return
                tiles, isint = pieces
                vws = (vsb, vbs)[:len(tiles)]
                for pi, (tl, vw) in enumerate(zip(tiles, vws)):
                    st = first and pi == 0
                    sp = last and pi == len(tiles) - 1
                    for hh in range(heads):
                        vbase = hh * NKC * (D + 1)
                        vch = vw[:, vbase + kc * (D + 1):
                                 vbase + (kc + 1) * (D + 1)]
                        rhs = tl[:, hh * QTILE:(hh + 1) * QTILE]
                        if isint:
                            rhs = rhs.bitcast(BF16)
                        nc.tensor.matmul(
                            poc[64 * hh:64 * hh + D + 1, :],
                            lhsT=vch,
                            rhs=rhs,
                            start=st,
                            stop=sp,
                            skip_group_check=True,
                            tile_position=(0, 64 * hh),
                        )

            def epilogue_steps(pocl, q0):
                def copy_step():
                    osb = osb_pool.tile([97, QTILE], F32, tag="osb",
                                        name="osb", uniquify=True)
                    osbs[0] = osb
                    for hh in range(heads):
                        ib = 64 * hh
                        nc.vector.tensor_copy(
                            osb[ib:ib + D + 1, :], pocl[ib:ib + D + 1, :]
                        )
                def norm_step(j):
                    # both heads -> one [128, 64] tile -> one contiguous DMA
                    outsb = outsb_pool.tile([P, COLS], F32, tag="outsb")
                    for hh in range(heads):
                        ib = 64 * hh
                        pt2 = ps_pool.tile([P, D + 1], F32, tag="ps",
                                           name="pt2", uniquify=True)
                        nc.tensor.transpose(
                            pt2[:], osbs[0][ib:ib + D + 1, j * P:(j + 1) * P],
                            ident[ib:ib + D + 1, ib:ib + D + 1],
                        )
                        rec = rec_pool.tile([P, 1], F32, tag="rec")
                        nc.vector.reciprocal(rec[:], pt2[:, D:D + 1])
                        nc.vector.tensor_scalar_mul(
                            outsb[:, D * hh:D * (hh + 1)], pt2[:, 0:D], rec[:]
                        )
                    nc.sync.dma_start(
                        out[q0 + j * P:q0 + (j + 1) * P, :], outsb[:]
                    )
                osbs = {}
                steps = [copy_step]
                for j in range(QTILE // P):
                    steps.append(lambda j=j: norm_step(j))
                return steps

            pending = []          # deferred epilogue of the previous q-tile
            pvq = []              # (pieces, kc, poc, q0) across q-tiles

            def pop_pv():
                pieces, kc, poc, q0 = pvq.pop(0)
                emit_pv(poc, pieces, kc, kc == 0, kc == NKC - 1)
                if kc == NKC - 1:
                    # this q-tile's accumulation is complete; queue its
                    # epilogue to be spread over the following groups
                    pending.extend(epilogue_steps(poc, q0))

            for qt in range(NQT):
                q0 = qt * QTILE
                poc = po_pool.tile([97, QTILE], F32, tag="po",
                                   name=f"po_{qt}")
                for g in range(NG):
                    psA, psB = emit_s_group(qt, g)
                    dveA, dveB = sched[g]
                    pexpA = emit_exp(psA, dveA)
                    pexpB = emit_exp(psB, dveB)
                    # epilogue pops AFTER the exp emission so their DVE ops
                    # queue behind ts1 (not in front of it)
                    if pending:
                        pending.pop(0)()
                    if pending:
                        pending.pop(0)()
                    pvq.append((pexpA, 2 * g, poc, q0))
                    pvq.append((pexpB, 2 * g + 1, poc, q0))
                    # taper the skew at the very end so the tail drains
                    # alongside the last groups instead of after them
                    thr = pv_skew
                    if qt == NQT - 1:
                        thr = min(pv_skew, 2 * (NG - 1 - g))
                    while len(pvq) > thr:
                        pop_pv()
            while pvq:
                pop_pv()
            for step in pending:
                step()


def _build(loop=0, **emit_kw):
    """loop=0: production build.  loop>=1: body wrapped in an on-device
    For_i repeat loop (timing-only builds).  emit_kw: ablation knobs."""
    key = ("nc", loop, tuple(sorted(emit_kw.items())))
    if key in _cache:
        return _cache[key]
    nc = bacc.Bacc(
        "TRN2",
        target_bir_lowering=False,
        debug=False,
        enable_asserts=False,
        num_devices=NCORES,
    )
    q = nc.dram_tensor("q", [N, COLS], F32, kind="ExternalInput").ap()
    k = nc.dram_tensor("k", [N, COLS], F32, kind="ExternalInput").ap()
    v = nc.dram_tensor("v", [N, COLS], F32, kind="ExternalInput").ap()
    out = nc.dram_tensor("out", [N, COLS], F32, kind="ExternalOutput").ap()
    with tile.TileContext(nc) as tc:
        if loop:
            with tc.For_i(0, loop, 1):
                _emit(tc, nc, q, k, v, out, **emit_kw)
        else:
            _emit(tc, nc, q, k, v, out, **emit_kw)
    nc.compile()
    _cache[key] = nc
    return nc


def _in_maps(q, k, v):
    maps = []
    for c in range(NCORES):
        b, hp = divmod(c, 4)
        cs = slice(hp * COLS, (hp + 1) * COLS)
        maps.append({
            "q": np.ascontiguousarray(q[b, :, cs], dtype=np.float32),
            "k": np.ascontiguousarray(k[b, :, cs], dtype=np.float32),
            "v": np.ascontiguousarray(v[b, :, cs], dtype=np.float32),
        })
    return maps


def _assemble(results):
    out = np.empty((B, N, C), np.float32)
    for c in range(NCORES):
        b, hp = divmod(c, 4)
        out[b, :, hp * COLS:(hp + 1) * COLS] = results[c]["out"]
    return out


def kernel(q, k, v):
    nc = _build()
    res = bass_utils.run_bass_kernel_spmd(
        nc, _in_maps(q, k, v), core_ids=list(range(NCORES))
    )
    return _assemble(res.results)


# revision 24
# speedup vs baseline: 1.2313x; 1.2313x over previous
"""Multi-head attention kernel for Trainium2, sharded over 8 NeuronCores.

Problem: q,k,v [2, 4096, 256], 8 heads of d=32.  b*h = 16 head-instances
are sharded 2-per-core (core c -> batch c//4, head-pair c%4); no
cross-core communication.

Per-core algorithm (n=4096, d=32, 2 heads, fp16 operands / fp32 PSUM):

  phase 0: DMA q/k slabs [4096,64] TWICE side-by-side -> staged [128,128]
    row-chunks; PE-transpose to kT/qTt [128, n] with the head-pair
    duplicated on partitions 64-127 (d rows: h0 @0-31, h1 @32-63, dup
    @64-127).  The duplicate feeds PE-array row-tiles (64,0)/(96,0).
    PSUM->SBUF copies alternate ScalarE/DVE.  V stored as per-k-chunk
    [128, 33] tiles with a ones-column appended (softmax denominator
    rides the PV matmul for free).

  main loop (q-tiles of 512, groups of 2 k-chunks of 128):
    - S^T for 2 heads x 2 k-chunks as FOUR concurrent 32-row PE tiles
      (tile_position (0,0),(32,0),(64,0),(96,0)) -> two PSUM groups
      [128, 2x512]; 100% PE row utilization (contraction d=32).
    - exp: split across TWO engines per a per-qt schedule:
        ScalarE chunks: one ACTIVATE Exp [128,1024] PSUM->SBUF fp16.
        DVE chunks: Schraudolph pair-trick entirely on VectorE:
          i  = round(S * (scale*1024*log2e) + B)   int16  (1 elem/cyc)
          i2 = i + 512                                     (4x mode)
          P  = bitcast_fp16(i2) * 2^-0.5 + bitcast_fp16(i) (2x/4x mode)
        = exp(S*scale) * (1 +- 0.8%); the mean ratio is folded into B so
        DVE chunks match ScalarE chunks' scale.  Softmax normalization
        makes the shared bias exactly cancel.
    - O^T[33,512] += [V|1]^T P per head, 2-way col-tiled PV matmuls
      ((0,0)/(0,64)) accumulated in one PSUM bank [97, 512]; row 32/96
      collects the denominator via the ones-column.  PV skewed one group
      behind S so the PE streams S(g+1) while exp(g) runs.
    - per-q-tile epilogue (PE-transpose back, multiply by reciprocal
      denominator, DMA out) is spread across the next q-tile's groups.
"""

import numpy as np

import concourse.mybir as mybir
import concourse.tile as tile
from concourse import bacc, bass_utils
from concourse.masks import make_identity

B, N, C, H, D = 2, 4096, 256, 8, 32
NCORES = 8
HPC = 2                      # heads per core
COLS = HPC * D               # 64 per-core channel columns
P = 128                      # partitions / k-chunk
QTILE = 512                  # q columns per head per PSUM group
NKC = N // P                 # 32 k-chunks
NQT = N // QTILE             # 8 q-tiles per head
NG = NKC // 2                # 16 k-chunk groups (4-way S tiling)
SCALE = float(1.0 / np.sqrt(D))
F32 = mybir.dt.float32
BF16 = mybir.dt.float16  # fp16: same PE speed as bf16, 4x finer mantissa
I16 = mybir.dt.int16

# Schraudolph pair-trick constants (see docstring).  i = round(A*x + Bc),
# P = fp16_bits(i) + 2^-0.5 * fp16_bits(i+512) approximates
# MEANR * exp(x*SCALE); log2(MEANR) is subtracted from the bias so the
# result matches ScalarE chunks' exp() scale.
LOG2E = float(1.0 / np.log(2.0))
AEXP = float(SCALE * 1024.0 * LOG2E)
MEANR = 2.0813747       # E[pair/exp] at zero bias shift; measured, mean->1
BEXP = float(1024.0 * 15.0 - 1024.0 * np.log2(MEANR))
HALF = float(2.0 ** -0.5)

_cache = {}


def _dve_sched(dve_per_qt):
    """Bresenham spread of DVE-exp chunks over the 16 groups of a q-tile.
    Returns per-group tuple (kcA_dve, kcB_dve); kcA prefers ScalarE."""
    sched = []
    acc = 0
    for g in range(NG):
        n0 = (g * dve_per_qt) // NG
        n1 = ((g + 1) * dve_per_qt) // NG
        take = n1 - n0          # 0, 1 (or 2 when dve_per_qt > 16)
        sched.append((take >= 2, take >= 1))
    return sched


def _emit(tc, nc, q, k, v, out, heads=HPC, dve_per_qt=11, do_s=True,
          do_exp=True, do_pv=True, do_main=True, pv_skew=6):
    sched = _dve_sched(dve_per_qt)
    with tc.tile_pool(name="persist", bufs=1) as persist:
        ident = persist.tile([P, P], F32, name="ident")
        make_identity(nc, ident[:])
        identh = persist.tile([P, P], BF16, name="identh")
        nc.vector.tensor_copy(identh[:], ident[:])
        # kT/qTt duplicated layout: partitions 0-63 = heads, 64-127 = copy
        kT = persist.tile([P, N], BF16, name="kT")
        qTt = [persist.tile([P, QTILE], BF16, name=f"qTt{i}")
               for i in range(NQT)]
        # V with ones column: per head, 32 chunks of [128, 33].  vbs is
        # the same scaled by 2^-0.5 — the second Schraudolph pair piece's
        # weight, so the pair-sum rides the PV PSUM accumulation.
        vsb = persist.tile([P, HPC * NKC * (D + 1)], BF16, name="vsb")
        vbs = persist.tile([P, HPC * NKC * (D + 1)], BF16, name="vbs")

        NST = 4                      # staging quarters (whole-tile dep unit)
        CPQ = NKC // NST             # 8 row-chunks per quarter
        with tc.tile_pool(name="stage", bufs=1) as stage_pool:
            def quarter_dma(src, name):
                # each row-chunk lands contiguously via one HBM DMA, then
                # DVE converts to fp16 staged as [128, 128]-blocks: cols
                # 0-63 data, 64-127 duplicate (feeds upper PE row-tiles
                # after transpose).  fp16 makes the PE transposes 1 cyc/row
                # instead of 2.
                tiles = []
                for g in range(NST):
                    stf = stage_pool.tile([P, CPQ * COLS], F32,
                                          name=f"{name}f{g}")
                    st = stage_pool.tile([P, CPQ * 2 * COLS], BF16,
                                         name=f"{name}{g}")
                    sv = st[:].rearrange("p (i two d) -> p i two d", two=2,
                                         d=COLS)
                    svf = stf[:].rearrange("p (i d) -> p i d", d=COLS)
                    nc.sync.dma_start(
                        svf,
                        src.rearrange("(i p) d -> i p d", p=P)[
                            g * CPQ:(g + 1) * CPQ].rearrange("i p d -> p i d"),
                    )
                    # fp32->fp16 cast on DVE; the duplicate halves via a
                    # SBUF->SBUF DMA (off-engine)
                    nc.vector.tensor_copy(sv[:, :, 0, :], svf)
                    nc.sync.dma_start(sv[:, :, 1, :], sv[:, :, 0, :])
                    tiles.append(st)
                return tiles

            # DMA queue order: k first (needed in full before any S),
            # then q quarter by quarter, V last (only PV needs it).
            kst = quarter_dma(k, "kst")
            qst = quarter_dma(q, "qst")
            vstage = stage_pool.tile([P, NKC * COLS], F32, name="vstage")
            nc.sync.dma_start(
                vstage[:].rearrange("p (i d) -> p i d", d=COLS),
                v.rearrange("(i p) d -> p i d", p=P),
            )

            with tc.tile_pool(name="tp", bufs=4, space="PSUM") as tp:
                # batches of 4 row-chunk transposes -> one [128,512] copy;
                # copies alternate DVE / ScalarE (both idle in phase 0).
                def transpose_in(tiles, dst_of_batch, name):
                    for j in range(NKC // 4):
                        pt = tp.tile([P, 4 * P], BF16, tag="pt")
                        for jj in range(4):
                            i = 4 * j + jj
                            g, ii = divmod(i, CPQ)
                            nc.tensor.transpose(
                                pt[:, jj * P:(jj + 1) * P],
                                tiles[g][:, ii * 2 * COLS:(ii + 1) * 2 * COLS],
                                identh[:],
                            )
                        dst = dst_of_batch(j)
                        if j % 2 == 0:
                            nc.vector.tensor_copy(dst, pt[:])
                        else:
                            nc.scalar.activation(
                                dst, pt[:],
                                mybir.ActivationFunctionType.Copy,
                            )

                transpose_in(kst, lambda j: kT[:, j * 512:(j + 1) * 512],
                             "kT")
                transpose_in(qst, lambda j: qTt[j][:], "qT")

                vv = vsb[:].rearrange("p (hh i e) -> p hh i e",
                                      hh=HPC, e=D + 1)
                vst = vstage[:].rearrange("p (i d) -> p i d", d=COLS)
                for hh in range(HPC):
                    nc.vector.tensor_copy(
                        vv[:, hh, :, 0:D], vst[:, :, hh * D:(hh + 1) * D]
                    )
                onescol = persist.tile([P, HPC * NKC], F32, name="onescol")
                nc.vector.memset(onescol[:], 1.0)
                nc.vector.tensor_copy(
                    vv[:, :, :, D],
                    onescol[:].rearrange("p (hh i) -> p hh i", hh=HPC),
                )
                nc.vector.tensor_scalar_mul(vbs[:], vsb[:], HALF)

        if not do_main:
            return
        with (
            tc.tile_pool(name="ps", bufs=3, space="PSUM") as ps_pool,
            tc.tile_pool(name="po", bufs=2, space="PSUM") as po_pool,
            tc.tile_pool(name="pexp", bufs=8) as pexp_pool,
            tc.tile_pool(name="pint", bufs=4) as pint_pool,
            tc.tile_pool(name="pint2", bufs=4) as pint2_pool,
            tc.tile_pool(name="osb", bufs=2) as osb_pool,
            tc.tile_pool(name="rec", bufs=3) as rec_pool,
            tc.tile_pool(name="outsb", bufs=3) as outsb_pool,
        ):
            if not do_s:
                ps_fix = ps_pool.tile([P, HPC * QTILE], F32, tag="ps")
                nc.vector.memset(ps_fix[:], 0.25)
            if not do_exp:
                pexp_fix = pexp_pool.tile([P, HPC * QTILE], BF16, tag="pexp")
                nc.vector.memset(pexp_fix[:], 0.5)

            def emit_s_group(qt, g):
                """4-way row-tiled S^T for chunks (2g, 2g+1), both heads."""
                if not do_s:
                    return (ps_fix, ps_fix)
                psA = ps_pool.tile([P, HPC * QTILE], F32, tag="ps")
                psB = ps_pool.tile([P, HPC * QTILE], F32, tag="ps")
                kcA, kcB = 2 * g, 2 * g + 1
                for hh in range(heads):
                    rp = slice(D * hh, D * (hh + 1))
                    nc.tensor.matmul(
                        psA[:, hh * QTILE:(hh + 1) * QTILE],
                        lhsT=kT[rp, kcA * P:(kcA + 1) * P],
                        rhs=qTt[qt][rp, :],
                        start=True, stop=True,
                        tile_position=(D * hh, 0),
                    )
                for hh in range(heads):
                    rp = slice(64 + D * hh, 64 + D * (hh + 1))
                    nc.tensor.matmul(
                        psB[:, hh * QTILE:(hh + 1) * QTILE],
                        lhsT=kT[rp, kcB * P:(kcB + 1) * P],
                        rhs=qTt[qt][rp, :],
                        start=True, stop=True,
                        tile_position=(64 + D * hh, 0),
                    )
                return (psA, psB)

            def emit_exp(ps, use_dve):
                """exp of one [128, 2*512] S chunk.  ScalarE: one fp16 tile.
                DVE: two int16 Schraudolph pieces (summed later on the PE)."""
                if not do_exp:
                    return ((pexp_fix,), False)
                if not use_dve:
                    pexp = pexp_pool.tile([P, HPC * QTILE], BF16, tag="pexp")
                    nc.scalar.activation(
                        pexp[:], ps[:], mybir.ActivationFunctionType.Exp,
                        scale=SCALE,
                    )
                    return ((pexp,), False)
                pint = pint_pool.tile([P, HPC * QTILE], I16, tag="pint")
                nc.vector.tensor_scalar(
                    pint[:], ps[:], AEXP, BEXP,
                    mybir.AluOpType.mult, mybir.AluOpType.add,
                )
                pint2 = pint2_pool.tile([P, HPC * QTILE], I16, tag="pint2")
                nc.vector.tensor_scalar_add(pint2[:], pint[:], 512)
                return ((pint, pint2), True)

            def emit_pv(poc, pieces, kc, first, last):
                if not do_pv:
                    return
                tiles, isint = pieces
                vws = (vsb, vbs)[:len(tiles)]
                for pi, (tl, vw) in enumerate(zip(tiles, vws)):
                    st = first and pi == 0
                    sp = last and pi == len(tiles) - 1
                    for hh in range(heads):
                        vbase = hh * NKC * (D + 1)
                        vch = vw[:, vbase + kc * (D + 1):
                                 vbase + (kc + 1) * (D + 1)]
                        rhs = tl[:, hh * QTILE:(hh + 1) * QTILE]
                        if isint:
                            rhs = rhs.bitcast(BF16)
                        nc.tensor.matmul(
                            poc[64 * hh:64 * hh + D + 1, :],
                            lhsT=vch,
                            rhs=rhs,
                            start=st,
                            stop=sp,
                            skip_group_check=True,
                            tile_position=(0, 64 * hh),
                        )

            def epilogue_steps(pocl, q0):
                def copy_step():
                    osb = osb_pool.tile([97, QTILE], F32, tag="osb",
                                        name="osb", uniquify=True)
                    osbs[0] = osb
                    for hh in range(heads):
                        ib = 64 * hh
                        nc.vector.tensor_copy(
                            osb[ib:ib + D + 1, :], pocl[ib:ib + D + 1, :]
                        )
                def norm_step(j):
                    # both heads -> one [128, 64] tile -> one contiguous DMA
                    outsb = outsb_pool.tile([P, COLS], F32, tag="outsb")
                    for hh in range(heads):
                        ib = 64 * hh
                        pt2 = ps_pool.tile([P, D + 1], F32, tag="ps",
                                           name="pt2", uniquify=True)
                        nc.tensor.transpose(
                            pt2[:], osbs[0][ib:ib + D + 1, j * P:(j + 1) * P],
                            ident[ib:ib + D + 1, ib:ib + D + 1],
                        )
                        rec = rec_pool.tile([P, 1], F32, tag="rec")
                        nc.vector.reciprocal(rec[:], pt2[:, D:D + 1])
                        nc.vector.tensor_scalar_mul(
                            outsb[:, D * hh:D * (hh + 1)], pt2[:, 0:D], rec[:]
                        )
                    nc.sync.dma_start(
                        out[q0 + j * P:q0 + (j + 1) * P, :], outsb[:]
                    )
                osbs = {}
                steps = [copy_step]
                for j in range(QTILE // P):
                    steps.append(lambda j=j: norm_step(j))
                return steps

            pending = []          # deferred epilogue of the previous q-tile
            pvq = []              # (pieces, kc, poc, q0) across q-tiles

            def pop_pv():
                pieces, kc, poc, q0 = pvq.pop(0)
                emit_pv(poc, pieces, kc, kc == 0, kc == NKC - 1)
                if kc == NKC - 1:
                    # this q-tile's accumulation is complete; queue its
                    # epilogue to be spread over the following groups
                    pending.extend(epilogue_steps(poc, q0))

            for qt in range(NQT):
                q0 = qt * QTILE
                poc = po_pool.tile([97, QTILE], F32, tag="po",
                                   name=f"po_{qt}")
                for g in range(NG):
                    psA, psB = emit_s_group(qt, g)
                    dveA, dveB = sched[g]
                    pexpA = emit_exp(psA, dveA)
                    pexpB = emit_exp(psB, dveB)
                    # epilogue pops AFTER the exp emission so their DVE ops
                    # queue behind ts1 (not in front of it)
                    if pending:
                        pending.pop(0)()
                    if pending:
                        pending.pop(0)()
                    pvq.append((pexpA, 2 * g, poc, q0))
                    pvq.append((pexpB, 2 * g + 1, poc, q0))
                    # taper the skew at the very end so the tail drains
                    # alongside the last groups instead of after them
                    thr = pv_skew
                    if qt == NQT - 1:
                        thr = min(pv_skew, 2 * (NG - 1 - g))
                    while len(pvq) > thr:
                        pop_pv()
            while pvq:
                pop_pv()
            for step in pending:
                step()


def _build(loop=0, **emit_kw):
    """loop=0: production build.  loop>=1: body wrapped in an on-device
    For_i repeat loop (timing-only builds).  emit_kw: ablation knobs."""
    key = ("nc", loop, tuple(sorted(emit_kw.items())))
    if key in _cache:
        return _cache[key]
    nc = bacc.Bacc(
        "TRN2",
        target_bir_lowering=False,
        debug=False,
        enable_asserts=False,
        num_devices=NCORES,
    )
    q = nc.dram_tensor("q", [N, COLS], F32, kind="ExternalInput").ap()
    k = nc.dram_tensor("k", [N, COLS], F32, kind="ExternalInput").ap()
    v = nc.dram_tensor("v", [N, COLS], F32, kind="ExternalInput").ap()
    out = nc.dram_tensor("out", [N, COLS], F32, kind="ExternalOutput").ap()
    with tile.TileContext(nc) as tc:
        if loop:
            with tc.For_i(0, loop, 1):
                _emit(tc, nc, q, k, v, out, **emit_kw)
        else:
            _emit(tc, nc, q, k, v, out, **emit_kw)
    nc.compile()
    _cache[key] = nc
    return nc


def _in_maps(q, k, v):
    maps = []
    for c in range(NCORES):
        b, hp = divmod(c, 4)
        cs = slice(hp * COLS, (hp + 1) * COLS)
        maps.append({
            "q": np.ascontiguousarray(q[b, :, cs], dtype=np.float32),
            "k": np.ascontiguousarray(k[b, :, cs], dtype=np.float32),
            "v": np.ascontiguousarray(v[b, :, cs], dtype=np.float32),
        })
    return maps


def _assemble(results):
    out = np.empty((B, N, C), np.float32)
    for c in range(NCORES):
        b, hp = divmod(c, 4)
        out[b, :, hp * COLS:(hp + 1) * COLS] = results[c]["out"]
    return out


def kernel(q, k, v):
    nc = _build()
    res = bass_utils.run_bass_kernel_spmd(
        nc, _in_maps(q, k, v), core_ids=list(range(NCORES))
    )
    return _assemble(res.results)


# revision 25
# speedup vs baseline: 1.2735x; 1.0343x over previous
"""Multi-head attention kernel for Trainium2, sharded over 8 NeuronCores.

Problem: q,k,v [2, 4096, 256], 8 heads of d=32.  b*h = 16 head-instances
are sharded 2-per-core (core c -> batch c//4, head-pair c%4); no
cross-core communication.

Per-core algorithm (n=4096, d=32, 2 heads, fp16 operands / fp32 PSUM):

  phase 0: DMA q/k slabs [4096,64] TWICE side-by-side -> staged [128,128]
    row-chunks; PE-transpose to kT/qTt [128, n] with the head-pair
    duplicated on partitions 64-127 (d rows: h0 @0-31, h1 @32-63, dup
    @64-127).  The duplicate feeds PE-array row-tiles (64,0)/(96,0).
    PSUM->SBUF copies alternate ScalarE/DVE.  V stored as per-k-chunk
    [128, 33] tiles with a ones-column appended (softmax denominator
    rides the PV matmul for free).

  main loop (q-tiles of 512, groups of 2 k-chunks of 128):
    - S^T for 2 heads x 2 k-chunks as FOUR concurrent 32-row PE tiles
      (tile_position (0,0),(32,0),(64,0),(96,0)) -> two PSUM groups
      [128, 2x512]; 100% PE row utilization (contraction d=32).
    - exp: split across TWO engines per a per-qt schedule:
        ScalarE chunks: one ACTIVATE Exp [128,1024] PSUM->SBUF fp16.
        DVE chunks: Schraudolph pair-trick entirely on VectorE:
          i  = round(S * (scale*1024*log2e) + B)   int16  (1 elem/cyc)
          i2 = i + 512                                     (4x mode)
          P  = bitcast_fp16(i2) * 2^-0.5 + bitcast_fp16(i) (2x/4x mode)
        = exp(S*scale) * (1 +- 0.8%); the mean ratio is folded into B so
        DVE chunks match ScalarE chunks' scale.  Softmax normalization
        makes the shared bias exactly cancel.
    - O^T[33,512] += [V|1]^T P per head, 2-way col-tiled PV matmuls
      ((0,0)/(0,64)) accumulated in one PSUM bank [97, 512]; row 32/96
      collects the denominator via the ones-column.  PV skewed one group
      behind S so the PE streams S(g+1) while exp(g) runs.
    - per-q-tile epilogue (PE-transpose back, multiply by reciprocal
      denominator, DMA out) is spread across the next q-tile's groups.
"""

import numpy as np

import concourse.mybir as mybir
import concourse.tile as tile
from concourse import bacc, bass_utils
from concourse.masks import make_identity

B, N, C, H, D = 2, 4096, 256, 8, 32
NCORES = 8
HPC = 2                      # heads per core
COLS = HPC * D               # 64 per-core channel columns
P = 128                      # partitions / k-chunk
QTILE = 512                  # q columns per head per PSUM group
NKC = N // P                 # 32 k-chunks
NQT = N // QTILE             # 8 q-tiles per head
NG = NKC // 2                # 16 k-chunk groups (4-way S tiling)
SCALE = float(1.0 / np.sqrt(D))
F32 = mybir.dt.float32
BF16 = mybir.dt.float16  # fp16: same PE speed as bf16, 4x finer mantissa
I16 = mybir.dt.int16

# Schraudolph pair-trick constants (see docstring).  i = round(A*x + Bc),
# P = fp16_bits(i) + 2^-0.5 * fp16_bits(i+512) approximates
# MEANR * exp(x*SCALE); log2(MEANR) is subtracted from the bias so the
# result matches ScalarE chunks' exp() scale.
LOG2E = float(1.0 / np.log(2.0))
AEXP = float(SCALE * 1024.0 * LOG2E)
MEANR = 2.0813747       # E[pair/exp] at zero bias shift; measured, mean->1
BEXP = float(1024.0 * 15.0 - 1024.0 * np.log2(MEANR))
HALF = float(2.0 ** -0.5)

_cache = {}


def _dve_sched(dve_per_qt):
    """Bresenham spread of DVE-exp chunks over the 16 groups of a q-tile.
    Returns per-group tuple (kcA_dve, kcB_dve); kcA prefers ScalarE."""
    sched = []
    acc = 0
    for g in range(NG):
        n0 = (g * dve_per_qt) // NG
        n1 = ((g + 1) * dve_per_qt) // NG
        take = n1 - n0          # 0, 1 (or 2 when dve_per_qt > 16)
        sched.append((take >= 2, take >= 1))
    return sched


def _emit(tc, nc, q, k, v, out, heads=HPC, dve_per_qt=11, do_s=True,
          do_exp=True, do_pv=True, do_main=True, pv_skew=6):
    sched = _dve_sched(dve_per_qt)
    with tc.tile_pool(name="persist", bufs=1) as persist:
        ident = persist.tile([P, P], F32, name="ident")
        make_identity(nc, ident[:])
        identh = persist.tile([P, P], BF16, name="identh")
        nc.vector.tensor_copy(identh[:], ident[:])
        # kT/qTt duplicated layout: partitions 0-63 = heads, 64-127 = copy
        kT = persist.tile([P, N], BF16, name="kT")
        qTt = [persist.tile([P, QTILE], BF16, name=f"qTt{i}")
               for i in range(NQT)]
        # V with ones column: per head, 32 chunks of [128, 33].  vbs is
        # the same scaled by 2^-0.5 — the second Schraudolph pair piece's
        # weight, so the pair-sum rides the PV PSUM accumulation.
        vsb = persist.tile([P, HPC * NKC * (D + 1)], BF16, name="vsb")
        vbs = persist.tile([P, HPC * NKC * (D + 1)], BF16, name="vbs")

        NST = 4                      # staging quarters (whole-tile dep unit)
        CPQ = NKC // NST             # 8 row-chunks per quarter
        with tc.tile_pool(name="stage", bufs=1) as stage_pool:
            def quarter_dma(src, name):
                # each row-chunk lands contiguously via one HBM DMA, then
                # DVE converts to fp16 staged as [128, 128]-blocks: cols
                # 0-63 data, 64-127 duplicate (feeds upper PE row-tiles
                # after transpose).  fp16 makes the PE transposes 1 cyc/row
                # instead of 2.
                tiles = []
                for g in range(NST):
                    stf = stage_pool.tile([P, CPQ * COLS], F32,
                                          name=f"{name}f{g}")
                    st = stage_pool.tile([P, CPQ * 2 * COLS], BF16,
                                         name=f"{name}{g}")
                    sv = st[:].rearrange("p (i two d) -> p i two d", two=2,
                                         d=COLS)
                    svf = stf[:].rearrange("p (i d) -> p i d", d=COLS)
                    nc.sync.dma_start(
                        svf,
                        src.rearrange("(i p) d -> i p d", p=P)[
                            g * CPQ:(g + 1) * CPQ].rearrange("i p d -> p i d"),
                    )
                    # fp32->fp16 cast on DVE; the fp16 duplicate copy runs
                    # at 4x DVE rate (2-byte SBUF->SBUF), ~200ns
                    nc.vector.tensor_copy(sv[:, :, 0, :], svf)
                    nc.vector.tensor_copy(sv[:, :, 1, :], sv[:, :, 0, :])
                    tiles.append(st)
                return tiles

            # DMA queue order: k first (needed in full before any S),
            # then q quarter by quarter, V last (only PV needs it).
            kst = quarter_dma(k, "kst")
            qst = quarter_dma(q, "qst")
            vstage = stage_pool.tile([P, NKC * COLS], F32, name="vstage")
            nc.sync.dma_start(
                vstage[:].rearrange("p (i d) -> p i d", d=COLS),
                v.rearrange("(i p) d -> p i d", p=P),
            )

            with tc.tile_pool(name="tp", bufs=4, space="PSUM") as tp:
                # batches of 4 row-chunk transposes -> one [128,512] copy;
                # copies alternate DVE / ScalarE (both idle in phase 0).
                def transpose_in(tiles, dst_of_batch, name):
                    for j in range(NKC // 4):
                        pt = tp.tile([P, 4 * P], BF16, tag="pt")
                        for jj in range(4):
                            i = 4 * j + jj
                            g, ii = divmod(i, CPQ)
                            nc.tensor.transpose(
                                pt[:, jj * P:(jj + 1) * P],
                                tiles[g][:, ii * 2 * COLS:(ii + 1) * 2 * COLS],
                                identh[:],
                            )
                        dst = dst_of_batch(j)
                        if j % 2 == 0:
                            nc.vector.tensor_copy(dst, pt[:])
                        else:
                            nc.scalar.activation(
                                dst, pt[:],
                                mybir.ActivationFunctionType.Copy,
                            )

                transpose_in(kst, lambda j: kT[:, j * 512:(j + 1) * 512],
                             "kT")
                transpose_in(qst, lambda j: qTt[j][:], "qT")

                vv = vsb[:].rearrange("p (hh i e) -> p hh i e",
                                      hh=HPC, e=D + 1)
                vst = vstage[:].rearrange("p (i d) -> p i d", d=COLS)
                for hh in range(HPC):
                    nc.vector.tensor_copy(
                        vv[:, hh, :, 0:D], vst[:, :, hh * D:(hh + 1) * D]
                    )
                onescol = persist.tile([P, HPC * NKC], F32, name="onescol")
                nc.vector.memset(onescol[:], 1.0)
                nc.vector.tensor_copy(
                    vv[:, :, :, D],
                    onescol[:].rearrange("p (hh i) -> p hh i", hh=HPC),
                )
                nc.vector.tensor_scalar_mul(vbs[:], vsb[:], HALF)

        if not do_main:
            return
        with (
            tc.tile_pool(name="ps", bufs=3, space="PSUM") as ps_pool,
            tc.tile_pool(name="po", bufs=2, space="PSUM") as po_pool,
            tc.tile_pool(name="pexp", bufs=8) as pexp_pool,
            tc.tile_pool(name="pint", bufs=4) as pint_pool,
            tc.tile_pool(name="pint2", bufs=4) as pint2_pool,
            tc.tile_pool(name="osb", bufs=2) as osb_pool,
            tc.tile_pool(name="rec", bufs=3) as rec_pool,
            tc.tile_pool(name="outsb", bufs=3) as outsb_pool,
        ):
            if not do_s:
                ps_fix = ps_pool.tile([P, HPC * QTILE], F32, tag="ps")
                nc.vector.memset(ps_fix[:], 0.25)
            if not do_exp:
                pexp_fix = pexp_pool.tile([P, HPC * QTILE], BF16, tag="pexp")
                nc.vector.memset(pexp_fix[:], 0.5)

            def emit_s_group(qt, g):
                """4-way row-tiled S^T for chunks (2g, 2g+1), both heads."""
                if not do_s:
                    return (ps_fix, ps_fix)
                psA = ps_pool.tile([P, HPC * QTILE], F32, tag="ps")
                psB = ps_pool.tile([P, HPC * QTILE], F32, tag="ps")
                kcA, kcB = 2 * g, 2 * g + 1
                for hh in range(heads):
                    rp = slice(D * hh, D * (hh + 1))
                    nc.tensor.matmul(
                        psA[:, hh * QTILE:(hh + 1) * QTILE],
                        lhsT=kT[rp, kcA * P:(kcA + 1) * P],
                        rhs=qTt[qt][rp, :],
                        start=True, stop=True,
                        tile_position=(D * hh, 0),
                    )
                for hh in range(heads):
                    rp = slice(64 + D * hh, 64 + D * (hh + 1))
                    nc.tensor.matmul(
                        psB[:, hh * QTILE:(hh + 1) * QTILE],
                        lhsT=kT[rp, kcB * P:(kcB + 1) * P],
                        rhs=qTt[qt][rp, :],
                        start=True, stop=True,
                        tile_position=(64 + D * hh, 0),
                    )
                return (psA, psB)

            def emit_exp(ps, use_dve):
                """exp of one [128, 2*512] S chunk.  ScalarE: one fp16 tile.
                DVE: two int16 Schraudolph pieces (summed later on the PE)."""
                if not do_exp:
                    return ((pexp_fix,), False)
                if not use_dve:
                    pexp = pexp_pool.tile([P, HPC * QTILE], BF16, tag="pexp")
                    nc.scalar.activation(
                        pexp[:], ps[:], mybir.ActivationFunctionType.Exp,
                        scale=SCALE,
                    )
                    return ((pexp,), False)
                pint = pint_pool.tile([P, HPC * QTILE], I16, tag="pint")
                nc.vector.tensor_scalar(
                    pint[:], ps[:], AEXP, BEXP,
                    mybir.AluOpType.mult, mybir.AluOpType.add,
                )
                pint2 = pint2_pool.tile([P, HPC * QTILE], I16, tag="pint2")
                nc.vector.tensor_scalar_add(pint2[:], pint[:], 512)
                return ((pint, pint2), True)

            def emit_pv(poc, pieces, kc, first, last):
                if not do_pv:
                    return
                tiles, isint = pieces
                vws = (vsb, vbs)[:len(tiles)]
                for pi, (tl, vw) in enumerate(zip(tiles, vws)):
                    st = first and pi == 0
                    sp = last and pi == len(tiles) - 1
                    for hh in range(heads):
                        vbase = hh * NKC * (D + 1)
                        vch = vw[:, vbase + kc * (D + 1):
                                 vbase + (kc + 1) * (D + 1)]
                        rhs = tl[:, hh * QTILE:(hh + 1) * QTILE]
                        if isint:
                            rhs = rhs.bitcast(BF16)
                        nc.tensor.matmul(
                            poc[64 * hh:64 * hh + D + 1, :],
                            lhsT=vch,
                            rhs=rhs,
                            start=st,
                            stop=sp,
                            skip_group_check=True,
                            tile_position=(0, 64 * hh),
                        )

            def epilogue_steps(pocl, q0):
                def copy_step():
                    osb = osb_pool.tile([97, QTILE], F32, tag="osb",
                                        name="osb", uniquify=True)
                    osbs[0] = osb
                    for hh in range(heads):
                        ib = 64 * hh
                        nc.vector.tensor_copy(
                            osb[ib:ib + D + 1, :], pocl[ib:ib + D + 1, :]
                        )
                def norm_step(j):
                    # both heads -> one [128, 64] tile -> one contiguous DMA
                    outsb = outsb_pool.tile([P, COLS], F32, tag="outsb")
                    for hh in range(heads):
                        ib = 64 * hh
                        pt2 = ps_pool.tile([P, D + 1], F32, tag="ps",
                                           name="pt2", uniquify=True)
                        nc.tensor.transpose(
                            pt2[:], osbs[0][ib:ib + D + 1, j * P:(j + 1) * P],
                            ident[ib:ib + D + 1, ib:ib + D + 1],
                        )
                        rec = rec_pool.tile([P, 1], F32, tag="rec")
                        nc.vector.reciprocal(rec[:], pt2[:, D:D + 1])
                        nc.vector.tensor_scalar_mul(
                            outsb[:, D * hh:D * (hh + 1)], pt2[:, 0:D], rec[:]
                        )
                    nc.sync.dma_start(
                        out[q0 + j * P:q0 + (j + 1) * P, :], outsb[:]
                    )
                osbs = {}
                steps = [copy_step]
                for j in range(QTILE // P):
                    steps.append(lambda j=j: norm_step(j))
                return steps

            pending = []          # deferred epilogue of the previous q-tile
            pvq = []              # (pieces, kc, poc, q0) across q-tiles

            def pop_pv():
                pieces, kc, poc, q0 = pvq.pop(0)
                emit_pv(poc, pieces, kc, kc == 0, kc == NKC - 1)
                if kc == NKC - 1:
                    # this q-tile's accumulation is complete; queue its
                    # epilogue to be spread over the following groups
                    pending.extend(epilogue_steps(poc, q0))

            for qt in range(NQT):
                q0 = qt * QTILE
                poc = po_pool.tile([97, QTILE], F32, tag="po",
                                   name=f"po_{qt}")
                for g in range(NG):
                    psA, psB = emit_s_group(qt, g)
                    dveA, dveB = sched[g]
                    pexpA = emit_exp(psA, dveA)
                    pexpB = emit_exp(psB, dveB)
                    # epilogue pops AFTER the exp emission so their DVE ops
                    # queue behind ts1 (not in front of it)
                    if pending:
                        pending.pop(0)()
                    if pending:
                        pending.pop(0)()
                    pvq.append((pexpA, 2 * g, poc, q0))
                    pvq.append((pexpB, 2 * g + 1, poc, q0))
                    # taper the skew at the very end so the tail drains
                    # alongside the last groups instead of after them
                    thr = pv_skew
                    if qt == NQT - 1:
                        thr = min(pv_skew, 2 * (NG - 1 - g))
                    while len(pvq) > thr:
                        pop_pv()
            while pvq:
                pop_pv()
            for step in pending:
                step()


def _build(loop=0, **emit_kw):
    """loop=0: production build.  loop>=1: body wrapped in an on-device
    For_i repeat loop (timing-only builds).  emit_kw: ablation knobs."""
    key = ("nc", loop, tuple(sorted(emit_kw.items())))
    if key in _cache:
        return _cache[key]
    nc = bacc.Bacc(
        "TRN2",
        target_bir_lowering=False,
        debug=False,
        enable_asserts=False,
        num_devices=NCORES,
    )
    q = nc.dram_tensor("q", [N, COLS], F32, kind="ExternalInput").ap()
    k = nc.dram_tensor("k", [N, COLS], F32, kind="ExternalInput").ap()
    v = nc.dram_tensor("v", [N, COLS], F32, kind="ExternalInput").ap()
    out = nc.dram_tensor("out", [N, COLS], F32, kind="ExternalOutput").ap()
    with tile.TileContext(nc) as tc:
        if loop:
            with tc.For_i(0, loop, 1):
                _emit(tc, nc, q, k, v, out, **emit_kw)
        else:
            _emit(tc, nc, q, k, v, out, **emit_kw)
    nc.compile()
    _cache[key] = nc
    return nc


def _in_maps(q, k, v):
    maps = []
    for c in range(NCORES):
        b, hp = divmod(c, 4)
        cs = slice(hp * COLS, (hp + 1) * COLS)
        maps.append({
            "q": np.ascontiguousarray(q[b, :, cs], dtype=np.float32),
            "k": np.ascontiguousarray(k[b, :, cs], dtype=np.float32),
            "v": np.ascontiguousarray(v[b, :, cs], dtype=np.float32),
        })
    return maps


def _assemble(results):
    out = np.empty((B, N, C), np.float32)
    for c in range(NCORES):
        b, hp = divmod(c, 4)
        out[b, :, hp * COLS:(hp + 1) * COLS] = results[c]["out"]
    return out


def kernel(q, k, v):
    nc = _build()
    res = bass_utils.run_bass_kernel_spmd(
        nc, _in_maps(q, k, v), core_ids=list(range(NCORES))
    )
    return _assemble(res.results)
